# revision 36
# baseline (speedup 1.0000x reference)
"""CaptionNet (attention + 2-LSTM) Trainium2 kernel, 8 NeuronCores.

Exploits:
- attention softmax over a size-1 axis == 1.0 exactly -> context == image_vectors
- LSTM2 uses h1 as input AND state -> W2 = W_ih2 + W_hh2 folds into one matmul
- word-emb / image parts of the LSTM1 input products are precomputed batched

Sharding: H sharded 8-way in the recurrent loop (h chunks exchanged via
AllGather twice/step); vocab projection sharded 8-way over V; weights
pre-transposed/pre-cast to bf16 on the host (pure layout/sharding work).

Perf structure (vs the naive version):
- AllGather output is pulled into SBUF as 8 per-k-chunk DMAs spread over
  4 engines' queues so the gate matmul starts ~0.4us after the AG lands
  and consumes chunks as they arrive.
- The SBUF->DRAM AG-input DMA is split across two engines' queues.
- The eltwise chain computes sigmoid(f) first so the DVE c-update starts
  while the other gate activations still run.
- pre-tiles (emb@W + img + b1) for step t+2 are computed inside step t's
  AG wait window instead of in the preamble.
- Dummy matmuls fill the PE idle window during each AG to keep the HAM
  clock gate at 8/8 (PE cold costs 2x on every matmul otherwise).
- Two throwaway AllGathers run during the preamble to warm the ncfw
  collective path before the first real (latency-critical) AG.
"""

import contextlib
import numpy as np
import ml_dtypes
import concourse.bass as bass
import concourse.mybir as mybir
from concourse.bass_utils import run_bass_kernel_spmd

B, T, V, E, H, F = 128, 24, 12000, 512, 1024, 2048
NC = 8
HC = H // NC          # 128
G = 4 * HC            # 512 gate rows per core (i,f,o,g reordered)
VC = V // NC          # 1500
KT = H // 128         # 8
ET = E // 128         # 4
FT = F // 128         # 16
F32 = mybir.dt.float32
BF16 = mybir.dt.bfloat16
AF = mybir.ActivationFunctionType
ALU = mybir.AluOpType
BF = ml_dtypes.bfloat16

# out-proj column split (PSUM banks are 512 f32 wide)
OPN = [(0, 512), (512, 512), (1024, VC - 1024)]
NTMR = 10    # DVE timer copies per AG window (~0.6us each) pacing the
NDUM = 3     # keep-warm dummy-matmul batches (NDUM matmuls per 2 timers)


def _kchunks(wT, n_free):
    """[K, n] -> [128, (K//128)*n]; [p, k*n+j] = wT[k*128+p, j]."""
    K = wT.shape[0]
    return np.ascontiguousarray(
        wT.reshape(K // 128, 128, n_free).transpose(1, 0, 2).reshape(128, -1))


def _build(nc):
    def inp(name, shape, dt):
        return nc.dram_tensor(name, list(shape), dt, kind="ExternalInput").ap()

    whh1T = inp("whh1T", [128, KT * G], BF16).rearrange("p (k g) -> p k g", k=KT)
    w2T = inp("w2T", [128, KT * G], BF16).rearrange("p (k g) -> p k g", k=KT)
    woutT = inp("woutT", [128, KT * VC], BF16).rearrange("p (k v) -> p k v", k=KT)
    weT = inp("weT", [128, ET * G], BF16).rearrange("p (k g) -> p k g", k=ET)
    wfT = inp("wfT", [128, FT * G], BF16).rearrange("p (k g) -> p k g", k=FT)
    wimgT = inp("wimgT", [128, FT * H], BF16).rearrange("p (k h) -> p k h", k=FT)
    wimgcT = inp("wimgcT", [128, FT * 128], BF16).rearrange(
        "p (k h) -> p k h", k=FT)
    ivT = inp("ivT", [128, FT * 128], BF16).rearrange("p (k b) -> p k b", k=FT)
    capT = inp("capT", [128, ET * B * T], BF16).rearrange(
        "p (k n) -> p k n", k=ET)
    b1b = inp("b1b", [128, G], BF16)
    b2b = inp("b2b", [128, G], BF16)
    bimgb = inp("bimgb", [128, H], BF16)
    bimgcb = inp("bimgcb", [128, 128], BF16)
    boutb = inp("boutb", [128, VC], BF16)
    idn = inp("idn", [128, 128], BF16)
    idnf = inp("idnf", [128, 128], F32)
    y = nc.dram_tensor("y", [B * T, VC], F32, kind="ExternalOutput").ap()
    # AG buffers declared flat so ncfw's SDMA descriptors cover large
    # contiguous runs (2D [128,256B] shapes made the AG desc-rate-bound).
    bin1 = nc.dram_tensor("bin1", [1, 128 * B], BF16, kind="Internal").ap()
    bout1 = nc.dram_tensor("bout1", [1, H * B], BF16, kind="Internal",
                           addr_space="Shared").ap()
    bin2 = nc.dram_tensor("bin2", [1, 128 * B], BF16, kind="Internal").ap()
    bout2 = nc.dram_tensor("bout2", [1, H * B], BF16, kind="Internal",
                           addr_space="Shared").ap()
    bin1v = bin1.rearrange("a (p b) -> (a p) b", b=B)          # [128, B]
    bin2v = bin2.rearrange("a (p b) -> (a p) b", b=B)
    bout1v = bout1.rearrange("a (k p b) -> (a p) k b", k=KT, b=B)  # [128,8,B]
    bout2v = bout2.rearrange("a (k p b) -> (a p) k b", k=KT, b=B)

    PE, ACT, DVE, SP, PL = nc.tensor, nc.scalar, nc.vector, nc.sync, nc.gpsimd
    ctx = contextlib.ExitStack()
    sb = lambda n, s, d: ctx.enter_context(nc.sbuf_tensor(n, s, d))
    ps = lambda n, s, d: ctx.enter_context(nc.psum_tensor(n, s, d))
    sem = lambda n: ctx.enter_context(nc.semaphore(n))

    # persistent SBUF
    s_whh1 = sb("s_whh1", [128, KT, G], BF16)
    s_w2 = sb("s_w2", [128, KT, G], BF16)
    s_wout = sb("s_wout", [128, KT, VC], BF16)
    s_pre = sb("s_pre", [128, T, G], BF16)
    s_cap = sb("s_cap", [128, ET, B * T], BF16)
    s_we = sb("s_we", [128, ET, G], BF16)
    s_img = sb("s_img", [128, G], BF16)
    s_b2b = sb("s_b2b", [128, G], BF16)
    s_boutb = sb("s_boutb", [128, VC], BF16)
    s_idn = sb("s_idn", [128, 128], BF16)
    s_idnf = sb("s_idnf", [128, 128], F32)
    s_h1T = sb("s_h1T", [128, KT, 128], BF16)
    s_h2T = [sb(f"s_h2T{i}", [128, KT, 128], BF16) for i in range(2)]
    s_c = sb("s_c", [128, HC], F32)
    s_sig = sb("s_sig", [128, 384], F32)
    s_tg = sb("s_tg", [128, HC], F32)
    s_th = sb("s_th", [128, HC], F32)
    s_tA = sb("s_tA", [128, HC], F32)
    s_tB = sb("s_tB", [128, HC], F32)
    s_h = sb("s_h", [128, HC], F32)
    s_hcT1 = sb("s_hcT1", [128, 128], BF16)
    s_hcT2 = sb("s_hcT2", [128, 128], BF16)
    s_out = sb("s_out", [128, VC], F32)
    s_tsrc = sb("s_tsrc", [128, 512], F32)  # DVE keep-warm timer scratch
    s_tdst = sb("s_tdst", [128, 512], F32)

    s_ld = sem("s_ld")
    s_warm = sem("s_warm")
    s_bh0 = sem("s_bh0"); s_bh0ev = sem("s_bh0ev")
    s_bimg = sem("s_bimg"); s_bimgev = sem("s_bimgev")
    s_bc0 = sem("s_bc0"); s_bc0ev = sem("s_bc0ev")
    s_bh0T = sem("s_bh0T"); s_bh0Tev = sem("s_bh0Tev")
    s_preMM = sem("s_preMM"); s_preEv = sem("s_preEv")
    s_gd = sem("s_gd")                      # g1/g2 drains, +2/step
    s_sf = sem("s_sf"); s_si = sem("s_si")  # eltwise ACT milestones, +2/step
    s_cc = sem("s_cc"); s_thS = sem("s_thS")
    s_hh = sem("s_hh")                      # h produced, +2/step
    s_tp = sem("s_tp"); s_ev = sem("s_ev")  # transpose + its evac, +2/step
    s_do = sem("s_do")                      # dma-out halves, +64/step
    s_ag1 = sem("s_ag1"); s_ag2 = sem("s_ag2")
    s_cSP = sem("s_cSP"); s_cPL = sem("s_cPL")  # pull sems (+16/phase each)
    s_cAC = sem("s_cAC")
    s_tmv = sem("s_tmv")  # DVE timer ticks, +2*NTMR/step
    s_op = sem("s_op"); s_oev = sem("s_oev"); s_odma = sem("s_odma")

    n_ld = 0
    def load(dst, src):
        nonlocal n_ld
        SP.dma_start(dst, src).then_inc(s_ld, 16)
        n_ld += 16

    load(s_whh1[:], whh1T)
    load(s_w2[:], w2T)
    load(s_wout[:], woutT)
    load(s_cap[:], capT)
    load(s_we[:], weT)
    load(s_b2b[:], b2b)
    load(s_boutb[:], boutb)
    load(s_idn[:], idn)
    load(s_idnf[:], idnf)

    # warm the ncfw collective path during the preamble (results unused)
    PL.collective_compute(
        "AllGather", ALU.bypass, replica_groups=[list(range(NC))],
        ins=[bin1.opt()], outs=[bout1.opt()]).then_inc(s_warm, 1)
    PL.collective_compute(
        "AllGather", ALU.bypass, replica_groups=[list(range(NC))],
        ins=[bin2.opt()], outs=[bout2.opt()]).then_inc(s_warm, 1)

    # AG output pulled 3 ways (k0-2 SP, k3-5 PL, k6-7 ACT); each engine
    # does one pull per AG phase: +32/step on its sem.
    def half_thresh(t, phase):
        return 32 * t + (32 if phase == 2 else 16)

    # ---------------- preamble ----------------
    with (
        nc.sbuf_tensor("s_wimg", [128, FT, H], BF16) as s_wimg,
        nc.sbuf_tensor("s_wimgc", [128, FT, 128], BF16) as s_wimgc,
        nc.sbuf_tensor("s_ivT", [128, FT, 128], BF16) as s_ivT,
        nc.sbuf_tensor("s_wf", [128, FT, G], BF16) as s_wf,
        nc.sbuf_tensor("s_b1b", [128, G], BF16) as s_b1b,
        nc.sbuf_tensor("s_bimgb", [128, H], BF16) as s_bimgb,
        nc.sbuf_tensor("s_bimgcb", [128, 128], BF16) as s_bimgcb,
        nc.sbuf_tensor("s_h0", [128, H], F32) as s_h0,
        nc.psum_tensor("p_h0", [128, H], F32) as p_h0,
        nc.psum_tensor("p_pre0", [128, 4, G], F32) as p_pre0,
    ):
        load(s_wimg[:], wimgT)
        load(s_wimgc[:], wimgcT)
        load(s_ivT[:], ivT)
        load(s_wf[:], wfT)
        load(s_b1b[:], b1b)
        load(s_bimgb[:], bimgb)
        load(s_bimgcb[:], bimgcb)
        PE.wait_ge(s_ld, n_ld)

        # h0 = IV @ W_img.T + b_img (replicated full)
        for nn2 in range(2):
            sl = slice(nn2 * 512, (nn2 + 1) * 512)
            for k in range(FT):
                PE.matmul(p_h0[:, sl], s_ivT[:, k, :], s_wimg[:, k, sl],
                          start=(k == 0), stop=False)
            PE.matmul(p_h0[:, sl], s_idn[:], s_bimgb[:, sl],
                      start=False, stop=True)
        PE.drain().then_inc(s_bh0, 1)
        DVE.wait_ge(s_bh0, 1)
        DVE.tensor_copy(s_h0[:], p_h0[:, :]).then_inc(s_bh0ev, 1)

        # c0 chunk = IV @ W_img[chunk].T + b_img[chunk]
        for k in range(FT):
            PE.matmul(p_pre0[:, 3, 0:128], s_ivT[:, k, :], s_wimgc[:, k, :],
                      start=(k == 0), stop=False)
        PE.matmul(p_pre0[:, 3, 0:128], s_idn[:], s_bimgcb[:],
                  start=False, stop=True)
        PE.drain().then_inc(s_bc0, 1)
        DVE.wait_ge(s_bc0, 1)
        DVE.tensor_copy(s_c[:], p_pre0[:, 3, 0:128]).then_inc(s_bc0ev, 1)

        # img_part = IV @ WF_c.T + b1  (bank 0)
        for k in range(FT):
            PE.matmul(p_pre0[:, 0, :], s_ivT[:, k, :], s_wf[:, k, :],
                      start=(k == 0), stop=False)
        PE.matmul(p_pre0[:, 0, :], s_idn[:], s_b1b[:], start=False, stop=True)
        PE.drain().then_inc(s_bimg, 1)
        ACT.wait_ge(s_bimg, 1)
        ACT.activation(s_img[:], p_pre0[:, 0, :], AF.Copy).then_inc(s_bimgev, 1)

        # h0T chunks -> s_h2T[1]  (bank 1, serialized via evac sem)
        PE.wait_ge(s_bh0ev, 1)
        for k in range(KT):
            if k > 0:
                PE.wait_ge(s_bh0Tev, k)
            PE.transpose(p_pre0[:, 1, 0:128], s_h0[:, k * 128:(k + 1) * 128],
                         s_idnf[:])
            PE.drain().then_inc(s_bh0T, 1)
            DVE.wait_ge(s_bh0T, k + 1)
            DVE.tensor_copy(s_h2T[1][:, k, :], p_pre0[:, 1, 0:128]).then_inc(
                s_bh0Tev, 1)

        # pre tiles 0 and 1 only (the rest are computed inside the T loop)
        PE.wait_ge(s_bimgev, 1)
        for m in range(2):
            bank = 0 if (m % 2 == 0) else 2
            for k in range(ET):
                PE.matmul(p_pre0[:, bank, :],
                          s_cap[:, k, m * 128:(m + 1) * 128],
                          s_we[:, k, :], start=(k == 0), stop=False)
            PE.matmul(p_pre0[:, bank, :], s_idn[:], s_img[:],
                      start=False, stop=True)
            PE.drain().then_inc(s_preMM, 1)
            if m % 2 == 0:
                ACT.wait_ge(s_preMM, m + 1)
                ACT.activation(s_pre[:, m, :], p_pre0[:, bank, :],
                               AF.Copy).then_inc(s_preEv, 1)
            else:
                DVE.wait_ge(s_preMM, m + 1)
                DVE.tensor_copy(s_pre[:, m, :], p_pre0[:, bank, :]).then_inc(
                    s_preEv, 1)

    # persistent PSUM
    p_g1 = ps("p_g1", [128, G], F32)
    p_g2 = ps("p_g2", [128, G], F32)
    p_t1 = ps("p_t1", [128, 128], F32)
    p_t2 = ps("p_t2", [128, 128], F32)
    p_op = ps("p_op", [128, 1536], F32)
    p_pre = ps("p_pre", [128, G], F32)

    def eltwise(psrc, phase_i):
        """phase_i = 2t+1 (lstm1) or 2t+2 (lstm2); reads psrc, updates s_c,
        writes s_h. f-gate sigmoid first so the DVE chain starts early."""
        ACT.wait_ge(s_gd, phase_i)
        ACT.activation(s_sig[:, 128:256], psrc[:, 128:256],
                       AF.Sigmoid).then_inc(s_sf, 1)
        ACT.activation(s_tg[:], psrc[:, 384:512], AF.Tanh)
        ACT.activation(s_sig[:, 0:128], psrc[:, 0:128],
                       AF.Sigmoid).then_inc(s_si, 1)
        ACT.activation(s_sig[:, 256:384], psrc[:, 256:384], AF.Sigmoid)
        DVE.wait_ge(s_sf, phase_i)
        DVE.scalar_tensor_tensor(s_tA[:], s_sig[:, 128:256], 1.0, s_c[:],
                                 ALU.mult, ALU.mult)
        DVE.wait_ge(s_si, phase_i)
        DVE.scalar_tensor_tensor(s_tB[:], s_sig[:, 0:128], 1.0, s_tg[:],
                                 ALU.mult, ALU.mult)
        DVE.scalar_tensor_tensor(s_c[:], s_tA[:], 1.0, s_tB[:],
                                 ALU.mult, ALU.add).then_inc(s_cc, 1)
        ACT.wait_ge(s_cc, phase_i)
        ACT.activation(s_th[:], s_c[:], AF.Tanh).then_inc(s_thS, 1)
        DVE.wait_ge(s_thS, phase_i)
        DVE.scalar_tensor_tensor(s_h[:], s_sig[:, 256:384], 1.0, s_th[:],
                                 ALU.mult, ALU.mult).then_inc(s_hh, 1)

    def op_chunks(h2Tprev, ks, start):
        for k in ks:
            for (o, w) in OPN:
                PE.matmul(p_op[:, o:o + w], h2Tprev[:, k, :],
                          s_wout[:, k, o:o + w],
                          start=(start and k == ks[0]), stop=False)

    for t in range(T):
        h2buf = s_h2T[(t - 1) % 2]   # holds h2(t-1); h0 lives in buf 1
        ph1, ph2 = 2 * t + 1, 2 * t + 2

        # ---- PE: g1(t), consuming AG2(t-1) pulls as they land ----
        for k in range(KT):
            if t > 0:
                if k == 0:
                    PE.wait_ge(s_cSP, half_thresh(t - 1, 2))
                elif k == 3:
                    PE.wait_ge(s_cPL, half_thresh(t - 1, 2))
                elif k == 6:
                    PE.wait_ge(s_cAC, half_thresh(t - 1, 2))
            elif k == 0:
                PE.wait_ge(s_bh0Tev, KT)
                PE.wait_ge(s_bc0ev, 1)
                PE.wait_ge(s_preEv, 1)
            PE.matmul(p_g1[:], h2buf[:, k, :], s_whh1[:, k, :],
                      start=(k == 0), stop=False)
        PE.matmul(p_g1[:], s_idn[:], s_pre[:, t, :], start=False, stop=True)
        PE.drain().then_inc(s_gd, 1)

        eltwise(p_g1, ph1)

        # ---- PE: out-proj(t-1) part 1 (covers the eltwise wait) ----
        if t > 0:
            PE.wait_ge(s_oev, t - 1)
            op_chunks(h2buf, [0, 1], start=True)

        # ---- PE: transpose h1 -> p_t1; ACT evacuates ----
        PE.wait_ge(s_hh, ph1)
        PE.transpose(p_t1[:], s_h[:], s_idnf[:])
        PE.drain().then_inc(s_tp, 1)
        ACT.wait_ge(s_tp, ph1)
        ACT.activation(s_hcT1[:], p_t1[:], AF.Copy).then_inc(s_ev, 1)

        # ---- SP/ACT/PL: 3-way AG1 input store; PL: doorbell ----
        SP.wait_ge(s_ev, ph1)
        if t > 0:
            SP.wait_ge(s_ag1, t)
        SP.dma_start(bin1v[0:64, :], s_hcT1[0:64, :]).then_inc(s_do, 16)
        if t > 0:
            ACT.wait_ge(s_ag1, t)
        ACT.dma_start(bin1v[64:96, :], s_hcT1[64:96, :]).then_inc(s_do, 16)
        PL.wait_ge(s_ev, ph1)
        PL.dma_start(bin1v[96:128, :], s_hcT1[96:128, :]).then_inc(s_do, 16)
        PL.wait_ge(s_do, 96 * t + 48)
        if t == 0:
            PL.wait_ge(s_warm, 2)
        PL.collective_compute(
            "AllGather", ALU.bypass, replica_groups=[list(range(NC))],
            ins=[bin1.opt()], outs=[bout1.opt()]).then_inc(s_ag1, 1)

        # ---- PE window 1 fill: rest of op A, pre(t+2), keep-warm ----
        if t > 0:
            op_chunks(h2buf, [2, 3], start=False)
        if t + 2 < T:
            PE.wait_ge(s_preEv, t + 2)  # WAW: previous tile evacuated
            for k in range(ET):
                PE.matmul(p_pre[:], s_cap[:, k, (t + 2) * 128:(t + 3) * 128],
                          s_we[:, k, :], start=(k == 0), stop=False)
            PE.matmul(p_pre[:], s_idn[:], s_img[:], start=False, stop=True)
            PE.drain().then_inc(s_preMM, 1)
            DVE.wait_ge(s_preMM, t + 3)  # 2 preamble drains + t+1 in-loop
            DVE.tensor_copy(s_pre[:, t + 2, :], p_pre[:]).then_inc(s_preEv, 1)
        # DVE timer chain paces keep-warm dummy matmuls through the AG wait
        for j in range(NTMR):
            DVE.tensor_copy(s_tdst[:], s_tsrc[:]).then_inc(s_tmv, 1)
        for j in range(NTMR // 2):
            PE.wait_ge(s_tmv, 2 * NTMR * t + 2 * (j + 1))
            for _ in range(NDUM):
                PE.matmul(p_t2[:], s_idn[:], s_idn[:], start=True, stop=True)

        # ---- AG1 pulls: k0-2 SP, k3-5 PL, k6-7 ACT ----
        SP.wait_ge(s_ag1, t + 1)
        SP.dma_start(s_h1T[:, 0:3, :], bout1v[:, 0:3, :]).then_inc(s_cSP, 16)
        PL.wait_ge(s_ag1, t + 1)
        PL.dma_start(s_h1T[:, 3:6, :], bout1v[:, 3:6, :]).then_inc(s_cPL, 16)
        ACT.wait_ge(s_ag1, t + 1)
        ACT.dma_start(s_h1T[:, 6:8, :], bout1v[:, 6:8, :]).then_inc(s_cAC, 16)

        # ---- PE: g2(t) ----
        for k in range(KT):
            if k == 0:
                PE.wait_ge(s_cSP, half_thresh(t, 1))
            elif k == 3:
                PE.wait_ge(s_cPL, half_thresh(t, 1))
            elif k == 6:
                PE.wait_ge(s_cAC, half_thresh(t, 1))
            PE.matmul(p_g2[:], s_h1T[:, k, :], s_w2[:, k, :],
                      start=(k == 0), stop=False)
        PE.matmul(p_g2[:], s_idn[:], s_b2b[:], start=False, stop=True)
        PE.drain().then_inc(s_gd, 1)

        eltwise(p_g2, ph2)

        # ---- PE: out-proj(t-1) part 2 start (covers eltwise2) ----
        if t > 0:
            op_chunks(h2buf, [4, 5], start=False)

        # ---- PE: transpose h2 -> p_t2; ACT evacuates ----
        PE.wait_ge(s_hh, ph2)
        PE.transpose(p_t2[:], s_h[:], s_idnf[:])
        PE.drain().then_inc(s_tp, 1)
        ACT.wait_ge(s_tp, ph2)
        ACT.activation(s_hcT2[:], p_t2[:], AF.Copy).then_inc(s_ev, 1)

        # ---- SP/ACT/PL: 3-way AG2 input store; PL: doorbell ----
        SP.wait_ge(s_ev, ph2)
        if t > 0:
            SP.wait_ge(s_ag2, t)
        SP.dma_start(bin2v[0:64, :], s_hcT2[0:64, :]).then_inc(s_do, 16)
        if t > 0:
            ACT.wait_ge(s_ag2, t)
        ACT.dma_start(bin2v[64:96, :], s_hcT2[64:96, :]).then_inc(s_do, 16)
        PL.wait_ge(s_ev, ph2)
        PL.dma_start(bin2v[96:128, :], s_hcT2[96:128, :]).then_inc(s_do, 16)
        PL.wait_ge(s_do, 96 * (t + 1))
        PL.collective_compute(
            "AllGather", ALU.bypass, replica_groups=[list(range(NC))],
            ins=[bin2.opt()], outs=[bout2.opt()]).then_inc(s_ag2, 1)

        # ---- PE window 2 fill: rest of op B + bias, keep-warm ----
        if t > 0:
            op_chunks(h2buf, [6, 7], start=False)
            for (o, w) in OPN:
                PE.matmul(p_op[:, o:o + w], s_idn[:], s_boutb[:, o:o + w],
                          start=False, stop=True)
            PE.drain().then_inc(s_op, 1)
            ACT.wait_ge(s_op, t)
            if t > 1:
                ACT.wait_ge(s_odma, 16 * (t - 1))
            ACT.activation(s_out[:], p_op[:, 0:VC], AF.Copy).then_inc(s_oev, 1)
            SP.wait_ge(s_oev, t)
            SP.dma_start(y[(t - 1) * 128:t * 128, :], s_out[:]).then_inc(
                s_odma, 16)
        for j in range(NTMR):
            DVE.tensor_copy(s_tdst[:], s_tsrc[:]).then_inc(s_tmv, 1)
        for j in range(NTMR // 2):
            PE.wait_ge(s_tmv, 2 * NTMR * t + NTMR + 2 * (j + 1))
            for _ in range(NDUM):
                PE.matmul(p_t1[:], s_idn[:], s_idn[:], start=True, stop=True)

        # ---- AG2 pulls into s_h2T[t % 2] ----
        dstbuf = s_h2T[t % 2]
        SP.wait_ge(s_ag2, t + 1)
        SP.dma_start(dstbuf[:, 0:3, :], bout2v[:, 0:3, :]).then_inc(s_cSP, 16)
        PL.wait_ge(s_ag2, t + 1)
        PL.dma_start(dstbuf[:, 3:6, :], bout2v[:, 3:6, :]).then_inc(s_cPL, 16)
        ACT.wait_ge(s_ag2, t + 1)
        ACT.dma_start(dstbuf[:, 6:8, :], bout2v[:, 6:8, :]).then_inc(s_cAC, 16)

    # epilogue: out-proj for t = T-1
    PE.wait_ge(s_oev, T - 1)
    PE.wait_ge(s_cSP, half_thresh(T - 1, 2))
    PE.wait_ge(s_cPL, half_thresh(T - 1, 2))
    PE.wait_ge(s_cAC, half_thresh(T - 1, 2))
    last = s_h2T[(T - 1) % 2]
    for k in range(KT):
        for (o, w) in OPN:
            PE.matmul(p_op[:, o:o + w], last[:, k, :], s_wout[:, k, o:o + w],
                      start=(k == 0), stop=False)
    for (o, w) in OPN:
        PE.matmul(p_op[:, o:o + w], s_idn[:], s_boutb[:, o:o + w],
                  start=False, stop=True)
    PE.drain().then_inc(s_op, 1)
    ACT.wait_ge(s_op, T)
    ACT.wait_ge(s_odma, 16 * (T - 1))
    ACT.activation(s_out[:], p_op[:, 0:VC], AF.Copy).then_inc(s_oev, 1)
    SP.wait_ge(s_oev, T)
    SP.dma_start(y[(T - 1) * 128:T * 128, :], s_out[:]).then_inc(s_odma, 16)
    SP.wait_ge(s_odma, 16 * T)

    ctx.close()
    return nc


def _in_maps(image_vectors, captions_ix, W_img, b_img, emb, Wa, ba, Ua, ub,
             va, vb, W_ih1, W_hh1, b_ih1, b_hh1, W_ih2, W_hh2, b_ih2, b_hh2,
             W_out, b_out):
    f32 = np.float32
    IV = np.asarray(image_vectors, f32)
    cap = np.asarray(captions_ix).astype(np.int64)
    W_ih1 = np.asarray(W_ih1, f32); W_hh1 = np.asarray(W_hh1, f32)
    W2 = np.asarray(W_ih2, f32) + np.asarray(W_hh2, f32)
    b1 = np.asarray(b_ih1, f32) + np.asarray(b_hh1, f32)
    b2 = np.asarray(b_ih2, f32) + np.asarray(b_hh2, f32)
    W_out = np.asarray(W_out, f32); b_out = np.asarray(b_out, f32)
    W_img = np.asarray(W_img, f32); b_img = np.asarray(b_img, f32)
    emb_ = np.asarray(emb, f32)

    def rows(c):
        idx = []
        for gsel in (0, 1, 3, 2):  # torch (i,f,g,o) -> ours (i,f,o,g)
            base = gsel * H + c * HC
            idx.extend(range(base, base + HC))
        return np.array(idx)

    ce = emb_[cap.T.reshape(-1)]                       # [(t,b), E]
    capT_np = _kchunks(ce.T.astype(BF), B * T)
    ivT_np = _kchunks(IV.T.astype(BF), 128)
    wimgT_np = _kchunks(W_img.T.astype(BF), H)
    bimgb_np = np.broadcast_to(b_img.astype(BF), (128, H)).copy()
    eye_bf = np.eye(128, dtype=f32).astype(BF)
    eye_f32 = np.eye(128, dtype=f32)

    in_maps = []
    for c in range(NC):
        r = rows(c)
        W1c = W_ih1[r]
        hr = slice(c * HC, (c + 1) * HC)
        in_maps.append({
            "whh1T": _kchunks(W_hh1[r].T.astype(BF), G),
            "w2T": _kchunks(W2[r].T.astype(BF), G),
            "woutT": _kchunks(W_out[c * VC:(c + 1) * VC].T.astype(BF), VC),
            "weT": _kchunks(W1c[:, :E].T.astype(BF), G),
            "wfT": _kchunks(W1c[:, E:].T.astype(BF), G),
            "wimgT": wimgT_np,
            "wimgcT": _kchunks(W_img[hr].T.astype(BF), 128),
            "ivT": ivT_np,
            "capT": capT_np,
            "b1b": np.broadcast_to(b1[r].astype(BF), (128, G)).copy(),
            "b2b": np.broadcast_to(b2[r].astype(BF), (128, G)).copy(),
            "bimgb": bimgb_np,
            "bimgcb": np.broadcast_to(b_img[hr].astype(BF), (128, 128)).copy(),
            "boutb": np.broadcast_to(
                b_out[c * VC:(c + 1) * VC].astype(BF), (128, VC)).copy(),
            "idn": eye_bf,
            "idnf": eye_f32,
        })
    return in_maps


def kernel(**inputs):
    in_maps = _in_maps(**inputs)
    nc = bass.Bass("TRN2", target_bir_lowering=False, debug=False,
                   num_devices=NC)
    _build(nc)
    res = run_bass_kernel_spmd(nc, in_maps, core_ids=list(range(NC)))

    out = np.empty((B, T, V), np.float32)
    for c in range(NC):
        yc = res.results[c]["y"].reshape(T, B, VC)
        out[:, :, c * VC:(c + 1) * VC] = yc.transpose(1, 0, 2)
    return out


# revision 45
# speedup vs baseline: 1.0437x; 1.0437x over previous
"""CaptionNet (attention + 2-LSTM) Trainium2 kernel, 8 NeuronCores.

Exploits:
- attention softmax over a size-1 axis == 1.0 exactly -> context == image_vectors
- LSTM2 uses h1 as input AND state -> W2 = W_ih2 + W_hh2 folds into one matmul
- word-emb / image parts of the LSTM1 input products are precomputed batched

Sharding: H sharded 8-way in the recurrent loop (h chunks exchanged via
AllGather twice/step); vocab projection sharded 8-way over V; weights
pre-transposed/pre-cast to bf16 on the host (pure layout/sharding work).

Perf structure (vs the naive version):
- AllGather output is pulled into SBUF as 8 per-k-chunk DMAs spread over
  4 engines' queues so the gate matmul starts ~0.4us after the AG lands
  and consumes chunks as they arrive.
- The SBUF->DRAM AG-input DMA is split across two engines' queues.
- The eltwise chain computes sigmoid(f) first so the DVE c-update starts
  while the other gate activations still run.
- pre-tiles (emb@W + img + b1) for step t+2 are computed inside step t's
  AG wait window instead of in the preamble.
- Dummy matmuls fill the PE idle window during each AG to keep the HAM
  clock gate at 8/8 (PE cold costs 2x on every matmul otherwise).
- Two throwaway AllGathers run during the preamble to warm the ncfw
  collective path before the first real (latency-critical) AG.
"""

import contextlib
import numpy as np
import ml_dtypes
import concourse.bass as bass
import concourse.mybir as mybir
from concourse.bass_utils import run_bass_kernel_spmd

B, T, V, E, H, F = 128, 24, 12000, 512, 1024, 2048
NC = 8
HC = H // NC          # 128
G = 4 * HC            # 512 gate rows per core (i,f,o,g reordered)
VC = V // NC          # 1500
KT = H // 128         # 8
ET = E // 128         # 4
FT = F // 128         # 16
F32 = mybir.dt.float32
BF16 = mybir.dt.bfloat16
AF = mybir.ActivationFunctionType
ALU = mybir.AluOpType
BF = ml_dtypes.bfloat16

# out-proj column split (PSUM banks are 512 f32 wide)
OPN = [(0, 512), (512, 512), (1024, VC - 1024)]
NTMR = 12    # DVE timer copies per AG window (~0.85us each) pacing the
NDUM = 3     # keep-warm dummy-matmul batches (NDUM matmuls per 2 timers)


def _kchunks(wT, n_free):
    """[K, n] -> [128, (K//128)*n]; [p, k*n+j] = wT[k*128+p, j]."""
    K = wT.shape[0]
    return np.ascontiguousarray(
        wT.reshape(K // 128, 128, n_free).transpose(1, 0, 2).reshape(128, -1))


def _build(nc):
    def inp(name, shape, dt):
        return nc.dram_tensor(name, list(shape), dt, kind="ExternalInput").ap()

    whh1T = inp("whh1T", [128, KT * G], BF16).rearrange("p (k g) -> p k g", k=KT)
    w2T = inp("w2T", [128, KT * G], BF16).rearrange("p (k g) -> p k g", k=KT)
    woutT = inp("woutT", [128, KT * VC], BF16).rearrange("p (k v) -> p k v", k=KT)
    weT = inp("weT", [128, ET * G], BF16).rearrange("p (k g) -> p k g", k=ET)
    wfT = inp("wfT", [128, FT * G], BF16).rearrange("p (k g) -> p k g", k=FT)
    wimgT = inp("wimgT", [128, FT * H], BF16).rearrange("p (k h) -> p k h", k=FT)
    wimgcT = inp("wimgcT", [128, FT * 128], BF16).rearrange(
        "p (k h) -> p k h", k=FT)
    ivT = inp("ivT", [128, FT * 128], BF16).rearrange("p (k b) -> p k b", k=FT)
    capT = inp("capT", [128, ET * B * T], BF16).rearrange(
        "p (k n) -> p k n", k=ET)
    b1b = inp("b1b", [128, G], BF16)
    b2b = inp("b2b", [128, G], BF16)
    bimgb = inp("bimgb", [128, H], BF16)
    bimgcb = inp("bimgcb", [128, 128], BF16)
    boutb = inp("boutb", [128, VC], BF16)
    idn = inp("idn", [128, 128], BF16)
    idnf = inp("idnf", [128, 128], F32)
    y = nc.dram_tensor("y", [B * T, VC], F32, kind="ExternalOutput").ap()
    # AG buffers declared flat so ncfw's SDMA descriptors cover large
    # contiguous runs (2D [128,256B] shapes made the AG desc-rate-bound).
    bin1 = nc.dram_tensor("bin1", [1, 128 * B], BF16, kind="Internal").ap()
    bout1 = nc.dram_tensor("bout1", [1, H * B], BF16, kind="Internal",
                           addr_space="Shared").ap()
    bin2 = nc.dram_tensor("bin2", [1, 128 * B], BF16, kind="Internal").ap()
    bout2 = nc.dram_tensor("bout2", [1, H * B], BF16, kind="Internal",
                           addr_space="Shared").ap()
    bin1v = bin1.rearrange("a (p b) -> (a p) b", b=B)          # [128, B]
    bin2v = bin2.rearrange("a (p b) -> (a p) b", b=B)
    bout1v = bout1.rearrange("a (k p b) -> (a p) k b", k=KT, b=B)  # [128,8,B]
    bout2v = bout2.rearrange("a (k p b) -> (a p) k b", k=KT, b=B)

    PE, ACT, DVE, SP, PL = nc.tensor, nc.scalar, nc.vector, nc.sync, nc.gpsimd
    ctx = contextlib.ExitStack()
    sb = lambda n, s, d: ctx.enter_context(nc.sbuf_tensor(n, s, d))
    ps = lambda n, s, d: ctx.enter_context(nc.psum_tensor(n, s, d))
    sem = lambda n: ctx.enter_context(nc.semaphore(n))

    # persistent SBUF
    s_whh1 = sb("s_whh1", [128, KT, G], BF16)
    s_w2 = sb("s_w2", [128, KT, G], BF16)
    s_wout = sb("s_wout", [128, KT, VC], BF16)
    s_pre = sb("s_pre", [128, T, G], BF16)
    s_cap = sb("s_cap", [128, ET, B * T], BF16)
    s_we = sb("s_we", [128, ET, G], BF16)
    s_img = sb("s_img", [128, G], BF16)
    s_b2b = sb("s_b2b", [128, G], BF16)
    s_boutb = sb("s_boutb", [128, VC], BF16)
    s_idn = sb("s_idn", [128, 128], BF16)
    s_idnf = sb("s_idnf", [128, 128], F32)
    s_h1T = sb("s_h1T", [128, KT, 128], BF16)
    s_h2T = [sb(f"s_h2T{i}", [128, KT, 128], BF16) for i in range(2)]
    s_c = sb("s_c", [128, HC], F32)
    s_sig = sb("s_sig", [128, 384], F32)
    s_tg = sb("s_tg", [128, HC], F32)
    s_th = sb("s_th", [128, HC], F32)
    s_tA = sb("s_tA", [128, HC], F32)
    s_tB = sb("s_tB", [128, HC], F32)
    s_h = sb("s_h", [128, HC], F32)
    s_hcT1 = sb("s_hcT1", [128, 128], BF16)
    s_hcT2 = sb("s_hcT2", [128, 128], BF16)
    s_out = sb("s_out", [128, VC], F32)
    s_tsrc = sb("s_tsrc", [128, 1024], F32)  # DVE keep-warm timer scratch
    s_tdst = sb("s_tdst", [128, 1024], F32)

    s_ld = sem("s_ld")
    s_warm = sem("s_warm")
    s_bh0 = sem("s_bh0"); s_bh0ev = sem("s_bh0ev")
    s_bimg = sem("s_bimg"); s_bimgev = sem("s_bimgev")
    s_bc0 = sem("s_bc0"); s_bc0ev = sem("s_bc0ev")
    s_bh0T = sem("s_bh0T"); s_bh0Tev = sem("s_bh0Tev")
    s_preMM = sem("s_preMM"); s_preEv = sem("s_preEv")
    s_gd = sem("s_gd")                      # g1/g2 drains, +2/step
    s_sf = sem("s_sf"); s_si = sem("s_si")  # eltwise ACT milestones, +2/step
    s_cc = sem("s_cc"); s_thS = sem("s_thS")
    s_hh = sem("s_hh")                      # h produced, +2/step
    s_tp = sem("s_tp"); s_ev = sem("s_ev")  # transpose + its evac, +2/step
    s_do = sem("s_do")                      # dma-out halves, +64/step
    s_ag1 = sem("s_ag1"); s_ag2 = sem("s_ag2")
    s_cSP = sem("s_cSP"); s_cAC = sem("s_cAC")  # pull sems (+16/phase each)
    s_tmv = sem("s_tmv")  # DVE timer ticks, +2*NTMR/step
    s_op = sem("s_op"); s_oev = sem("s_oev"); s_odma = sem("s_odma")

    n_ld = 0
    def load(dst, src):
        nonlocal n_ld
        SP.dma_start(dst, src).then_inc(s_ld, 16)
        n_ld += 16

    load(s_whh1[:], whh1T)
    load(s_w2[:], w2T)
    load(s_wout[:], woutT)
    load(s_cap[:], capT)
    load(s_we[:], weT)
    load(s_b2b[:], b2b)
    load(s_boutb[:], boutb)
    load(s_idn[:], idn)
    load(s_idnf[:], idnf)

    # warm the ncfw collective path during the preamble (results unused)
    PL.collective_compute(
        "AllGather", ALU.bypass, replica_groups=[list(range(NC))],
        ins=[bin1.opt()], outs=[bout1.opt()]).then_inc(s_warm, 1)
    PL.collective_compute(
        "AllGather", ALU.bypass, replica_groups=[list(range(NC))],
        ins=[bin2.opt()], outs=[bout2.opt()]).then_inc(s_warm, 1)

    # AG output pulled 3 ways (k0-2 SP, k3-5 PL, k6-7 ACT); each engine
    # does one pull per AG phase: +32/step on its sem.
    def half_thresh(t, phase):
        return 32 * t + (32 if phase == 2 else 16)

    # ---------------- preamble ----------------
    with (
        nc.sbuf_tensor("s_wimg", [128, FT, H], BF16) as s_wimg,
        nc.sbuf_tensor("s_wimgc", [128, FT, 128], BF16) as s_wimgc,
        nc.sbuf_tensor("s_ivT", [128, FT, 128], BF16) as s_ivT,
        nc.sbuf_tensor("s_wf", [128, FT, G], BF16) as s_wf,
        nc.sbuf_tensor("s_b1b", [128, G], BF16) as s_b1b,
        nc.sbuf_tensor("s_bimgb", [128, H], BF16) as s_bimgb,
        nc.sbuf_tensor("s_bimgcb", [128, 128], BF16) as s_bimgcb,
        nc.sbuf_tensor("s_h0", [128, H], F32) as s_h0,
        nc.psum_tensor("p_h0", [128, H], F32) as p_h0,
        nc.psum_tensor("p_pre0", [128, 4, G], F32) as p_pre0,
    ):
        load(s_wimg[:], wimgT)
        load(s_wimgc[:], wimgcT)
        load(s_ivT[:], ivT)
        load(s_wf[:], wfT)
        load(s_b1b[:], b1b)
        load(s_bimgb[:], bimgb)
        load(s_bimgcb[:], bimgcb)
        PE.wait_ge(s_ld, n_ld)

        # h0 = IV @ W_img.T + b_img (replicated full)
        for nn2 in range(2):
            sl = slice(nn2 * 512, (nn2 + 1) * 512)
            for k in range(FT):
                PE.matmul(p_h0[:, sl], s_ivT[:, k, :], s_wimg[:, k, sl],
                          start=(k == 0), stop=False)
            PE.matmul(p_h0[:, sl], s_idn[:], s_bimgb[:, sl],
                      start=False, stop=True)
        PE.drain().then_inc(s_bh0, 1)
        DVE.wait_ge(s_bh0, 1)
        DVE.tensor_copy(s_h0[:], p_h0[:, :]).then_inc(s_bh0ev, 1)

        # c0 chunk = IV @ W_img[chunk].T + b_img[chunk]
        for k in range(FT):
            PE.matmul(p_pre0[:, 3, 0:128], s_ivT[:, k, :], s_wimgc[:, k, :],
                      start=(k == 0), stop=False)
        PE.matmul(p_pre0[:, 3, 0:128], s_idn[:], s_bimgcb[:],
                  start=False, stop=True)
        PE.drain().then_inc(s_bc0, 1)
        DVE.wait_ge(s_bc0, 1)
        DVE.tensor_copy(s_c[:], p_pre0[:, 3, 0:128]).then_inc(s_bc0ev, 1)

        # img_part = IV @ WF_c.T + b1  (bank 0)
        for k in range(FT):
            PE.matmul(p_pre0[:, 0, :], s_ivT[:, k, :], s_wf[:, k, :],
                      start=(k == 0), stop=False)
        PE.matmul(p_pre0[:, 0, :], s_idn[:], s_b1b[:], start=False, stop=True)
        PE.drain().then_inc(s_bimg, 1)
        ACT.wait_ge(s_bimg, 1)
        ACT.activation(s_img[:], p_pre0[:, 0, :], AF.Copy).then_inc(s_bimgev, 1)

        # h0T chunks -> s_h2T[1]  (bank 1, serialized via evac sem)
        PE.wait_ge(s_bh0ev, 1)
        for k in range(KT):
            if k > 0:
                PE.wait_ge(s_bh0Tev, k)
            PE.transpose(p_pre0[:, 1, 0:128], s_h0[:, k * 128:(k + 1) * 128],
                         s_idnf[:])
            PE.drain().then_inc(s_bh0T, 1)
            DVE.wait_ge(s_bh0T, k + 1)
            DVE.tensor_copy(s_h2T[1][:, k, :], p_pre0[:, 1, 0:128]).then_inc(
                s_bh0Tev, 1)

        # pre tiles 0 and 1 only (the rest are computed inside the T loop)
        PE.wait_ge(s_bimgev, 1)
        for m in range(2):
            bank = 0 if (m % 2 == 0) else 2
            for k in range(ET):
                PE.matmul(p_pre0[:, bank, :],
                          s_cap[:, k, m * 128:(m + 1) * 128],
                          s_we[:, k, :], start=(k == 0), stop=False)
            PE.matmul(p_pre0[:, bank, :], s_idn[:], s_img[:],
                      start=False, stop=True)
            PE.drain().then_inc(s_preMM, 1)
            if m % 2 == 0:
                ACT.wait_ge(s_preMM, m + 1)
                ACT.activation(s_pre[:, m, :], p_pre0[:, bank, :],
                               AF.Copy).then_inc(s_preEv, 1)
            else:
                DVE.wait_ge(s_preMM, m + 1)
                DVE.tensor_copy(s_pre[:, m, :], p_pre0[:, bank, :]).then_inc(
                    s_preEv, 1)

    # persistent PSUM
    p_g1 = ps("p_g1", [128, G], F32)
    p_g2 = ps("p_g2", [128, G], F32)
    p_t1 = ps("p_t1", [128, 128], F32)
    p_t2 = ps("p_t2", [128, 128], F32)
    p_op = ps("p_op", [128, 1536], F32)
    p_pre = ps("p_pre", [128, G], F32)

    def eltwise(psrc, phase_i):
        """phase_i = 2t+1 (lstm1) or 2t+2 (lstm2); reads psrc, updates s_c,
        writes s_h. f-gate sigmoid first so the DVE chain starts early."""
        ACT.wait_ge(s_gd, phase_i)
        ACT.activation(s_sig[:, 128:256], psrc[:, 128:256],
                       AF.Sigmoid).then_inc(s_sf, 1)
        ACT.activation(s_tg[:], psrc[:, 384:512], AF.Tanh)
        ACT.activation(s_sig[:, 0:128], psrc[:, 0:128],
                       AF.Sigmoid).then_inc(s_si, 1)
        ACT.activation(s_sig[:, 256:384], psrc[:, 256:384], AF.Sigmoid)
        DVE.wait_ge(s_sf, phase_i)
        DVE.scalar_tensor_tensor(s_tA[:], s_sig[:, 128:256], 1.0, s_c[:],
                                 ALU.mult, ALU.mult)
        DVE.wait_ge(s_si, phase_i)
        DVE.scalar_tensor_tensor(s_tB[:], s_sig[:, 0:128], 1.0, s_tg[:],
                                 ALU.mult, ALU.mult)
        DVE.scalar_tensor_tensor(s_c[:], s_tA[:], 1.0, s_tB[:],
                                 ALU.mult, ALU.add).then_inc(s_cc, 1)
        ACT.wait_ge(s_cc, phase_i)
        ACT.activation(s_th[:], s_c[:], AF.Tanh).then_inc(s_thS, 1)
        DVE.wait_ge(s_thS, phase_i)
        DVE.scalar_tensor_tensor(s_h[:], s_sig[:, 256:384], 1.0, s_th[:],
                                 ALU.mult, ALU.mult).then_inc(s_hh, 1)

    def op_chunks(h2Tprev, ks, start):
        for k in ks:
            for (o, w) in OPN:
                PE.matmul(p_op[:, o:o + w], h2Tprev[:, k, :],
                          s_wout[:, k, o:o + w],
                          start=(start and k == ks[0]), stop=False)

    for t in range(T):
        h2buf = s_h2T[(t - 1) % 2]   # holds h2(t-1); h0 lives in buf 1
        ph1, ph2 = 2 * t + 1, 2 * t + 2

        # ---- PE: g1(t), consuming AG2(t-1) pulls as they land ----
        for k in range(KT):
            if t > 0:
                if k == 0:
                    PE.wait_ge(s_cSP, half_thresh(t - 1, 2))
                elif k == 4:
                    PE.wait_ge(s_cAC, half_thresh(t - 1, 2))
            elif k == 0:
                PE.wait_ge(s_bh0Tev, KT)
                PE.wait_ge(s_bc0ev, 1)
                PE.wait_ge(s_preEv, 1)
            PE.matmul(p_g1[:], h2buf[:, k, :], s_whh1[:, k, :],
                      start=(k == 0), stop=False)
        PE.matmul(p_g1[:], s_idn[:], s_pre[:, t, :], start=False, stop=True)
        PE.drain().then_inc(s_gd, 1)

        eltwise(p_g1, ph1)

        # ---- PE: out-proj(t-1) part 1 (covers the eltwise wait) ----
        if t > 0:
            PE.wait_ge(s_oev, t - 1)
            op_chunks(h2buf, [0, 1], start=True)

        # ---- PE: transpose h1 -> p_t1; ACT evacuates ----
        PE.wait_ge(s_hh, ph1)
        PE.transpose(p_t1[:], s_h[:], s_idnf[:])
        PE.drain().then_inc(s_tp, 1)
        ACT.wait_ge(s_tp, ph1)
        ACT.activation(s_hcT1[:], p_t1[:], AF.Copy).then_inc(s_ev, 1)

        # ---- SP + ACT: split AG1 input store; PL: doorbell ----
        SP.wait_ge(s_ev, ph1)
        if t > 0:
            SP.wait_ge(s_ag1, t)
        SP.dma_start(bin1v[0:64, :], s_hcT1[0:64, :]).then_inc(s_do, 16)
        if t > 0:
            ACT.wait_ge(s_ag1, t)
        ACT.dma_start(bin1v[64:128, :], s_hcT1[64:128, :]).then_inc(s_do, 16)
        PL.wait_ge(s_do, 64 * t + 32)
        if t == 0:
            PL.wait_ge(s_warm, 2)
        PL.collective_compute(
            "AllGather", ALU.bypass, replica_groups=[list(range(NC))],
            ins=[bin1.opt()], outs=[bout1.opt()]).then_inc(s_ag1, 1)

        # ---- PE window 1 fill: rest of op A, pre(t+2), keep-warm ----
        if t > 0:
            op_chunks(h2buf, [2, 3], start=False)
        if t + 2 < T:
            PE.wait_ge(s_preEv, t + 2)  # WAW: previous tile evacuated
            for k in range(ET):
                PE.matmul(p_pre[:], s_cap[:, k, (t + 2) * 128:(t + 3) * 128],
                          s_we[:, k, :], start=(k == 0), stop=False)
            PE.matmul(p_pre[:], s_idn[:], s_img[:], start=False, stop=True)
            PE.drain().then_inc(s_preMM, 1)
            DVE.wait_ge(s_preMM, t + 3)  # 2 preamble drains + t+1 in-loop
            DVE.tensor_copy(s_pre[:, t + 2, :], p_pre[:]).then_inc(s_preEv, 1)
        # DVE timer chain paces keep-warm dummy matmuls through the AG wait
        for j in range(NTMR):
            DVE.tensor_copy(s_tdst[:], s_tsrc[:]).then_inc(s_tmv, 1)
        for j in range(NTMR // 2):
            PE.wait_ge(s_tmv, 2 * NTMR * t + 2 * (j + 1))
            for _ in range(NDUM):
                PE.matmul(p_t2[:], s_idn[:], s_idn[:], start=True, stop=True)

        # ---- AG1 pulls: k0-3 SP, k4-7 ACT ----
        SP.wait_ge(s_ag1, t + 1)
        SP.dma_start(s_h1T[:, 0:4, :], bout1v[:, 0:4, :]).then_inc(s_cSP, 16)
        ACT.wait_ge(s_ag1, t + 1)
        ACT.dma_start(s_h1T[:, 4:8, :], bout1v[:, 4:8, :]).then_inc(s_cAC, 16)

        # ---- PE: g2(t) ----
        for k in range(KT):
            if k == 0:
                PE.wait_ge(s_cSP, half_thresh(t, 1))
            elif k == 4:
                PE.wait_ge(s_cAC, half_thresh(t, 1))
            PE.matmul(p_g2[:], s_h1T[:, k, :], s_w2[:, k, :],
                      start=(k == 0), stop=False)
        PE.matmul(p_g2[:], s_idn[:], s_b2b[:], start=False, stop=True)
        PE.drain().then_inc(s_gd, 1)

        eltwise(p_g2, ph2)

        # ---- PE: out-proj(t-1) part 2 start (covers eltwise2) ----
        if t > 0:
            op_chunks(h2buf, [4, 5], start=False)

        # ---- PE: transpose h2 -> p_t2; ACT evacuates ----
        PE.wait_ge(s_hh, ph2)
        PE.transpose(p_t2[:], s_h[:], s_idnf[:])
        PE.drain().then_inc(s_tp, 1)
        ACT.wait_ge(s_tp, ph2)
        ACT.activation(s_hcT2[:], p_t2[:], AF.Copy).then_inc(s_ev, 1)

        # ---- SP + ACT: split AG2 input store; PL: doorbell ----
        SP.wait_ge(s_ev, ph2)
        if t > 0:
            SP.wait_ge(s_ag2, t)
        SP.dma_start(bin2v[0:64, :], s_hcT2[0:64, :]).then_inc(s_do, 16)
        if t > 0:
            ACT.wait_ge(s_ag2, t)
        ACT.dma_start(bin2v[64:128, :], s_hcT2[64:128, :]).then_inc(s_do, 16)
        PL.wait_ge(s_do, 64 * (t + 1))
        PL.collective_compute(
            "AllGather", ALU.bypass, replica_groups=[list(range(NC))],
            ins=[bin2.opt()], outs=[bout2.opt()]).then_inc(s_ag2, 1)

        # ---- PE window 2 fill: rest of op B + bias, keep-warm ----
        if t > 0:
            op_chunks(h2buf, [6, 7], start=False)
            for (o, w) in OPN:
                PE.matmul(p_op[:, o:o + w], s_idn[:], s_boutb[:, o:o + w],
                          start=False, stop=True)
            PE.drain().then_inc(s_op, 1)
            ACT.wait_ge(s_op, t)
            if t > 1:
                ACT.wait_ge(s_odma, 16 * (t - 1))
            ACT.activation(s_out[:], p_op[:, 0:VC], AF.Copy).then_inc(s_oev, 1)
            SP.wait_ge(s_oev, t)
            SP.dma_start(y[(t - 1) * 128:t * 128, :], s_out[:]).then_inc(
                s_odma, 16)
        for j in range(NTMR):
            DVE.tensor_copy(s_tdst[:], s_tsrc[:]).then_inc(s_tmv, 1)
        for j in range(NTMR // 2):
            PE.wait_ge(s_tmv, 2 * NTMR * t + NTMR + 2 * (j + 1))
            for _ in range(NDUM):
                PE.matmul(p_t1[:], s_idn[:], s_idn[:], start=True, stop=True)

        # ---- AG2 pulls into s_h2T[t % 2] ----
        dstbuf = s_h2T[t % 2]
        SP.wait_ge(s_ag2, t + 1)
        SP.dma_start(dstbuf[:, 0:4, :], bout2v[:, 0:4, :]).then_inc(s_cSP, 16)
        ACT.wait_ge(s_ag2, t + 1)
        ACT.dma_start(dstbuf[:, 4:8, :], bout2v[:, 4:8, :]).then_inc(s_cAC, 16)

    # epilogue: out-proj for t = T-1
    PE.wait_ge(s_oev, T - 1)
    PE.wait_ge(s_cSP, half_thresh(T - 1, 2))
    PE.wait_ge(s_cAC, half_thresh(T - 1, 2))
    last = s_h2T[(T - 1) % 2]
    for k in range(KT):
        for (o, w) in OPN:
            PE.matmul(p_op[:, o:o + w], last[:, k, :], s_wout[:, k, o:o + w],
                      start=(k == 0), stop=False)
    for (o, w) in OPN:
        PE.matmul(p_op[:, o:o + w], s_idn[:], s_boutb[:, o:o + w],
                  start=False, stop=True)
    PE.drain().then_inc(s_op, 1)
    ACT.wait_ge(s_op, T)
    ACT.wait_ge(s_odma, 16 * (T - 1))
    ACT.activation(s_out[:], p_op[:, 0:VC], AF.Copy).then_inc(s_oev, 1)
    SP.wait_ge(s_oev, T)
    SP.dma_start(y[(T - 1) * 128:T * 128, :], s_out[:]).then_inc(s_odma, 16)
    SP.wait_ge(s_odma, 16 * T)

    ctx.close()
    return nc


def _in_maps(image_vectors, captions_ix, W_img, b_img, emb, Wa, ba, Ua, ub,
             va, vb, W_ih1, W_hh1, b_ih1, b_hh1, W_ih2, W_hh2, b_ih2, b_hh2,
             W_out, b_out):
    f32 = np.float32
    IV = np.asarray(image_vectors, f32)
    cap = np.asarray(captions_ix).astype(np.int64)
    W_ih1 = np.asarray(W_ih1, f32); W_hh1 = np.asarray(W_hh1, f32)
    W2 = np.asarray(W_ih2, f32) + np.asarray(W_hh2, f32)
    b1 = np.asarray(b_ih1, f32) + np.asarray(b_hh1, f32)
    b2 = np.asarray(b_ih2, f32) + np.asarray(b_hh2, f32)
    W_out = np.asarray(W_out, f32); b_out = np.asarray(b_out, f32)
    W_img = np.asarray(W_img, f32); b_img = np.asarray(b_img, f32)
    emb_ = np.asarray(emb, f32)

    def rows(c):
        idx = []
        for gsel in (0, 1, 3, 2):  # torch (i,f,g,o) -> ours (i,f,o,g)
            base = gsel * H + c * HC
            idx.extend(range(base, base + HC))
        return np.array(idx)

    ce = emb_[cap.T.reshape(-1)]                       # [(t,b), E]
    capT_np = _kchunks(ce.T.astype(BF), B * T)
    ivT_np = _kchunks(IV.T.astype(BF), 128)
    wimgT_np = _kchunks(W_img.T.astype(BF), H)
    bimgb_np = np.broadcast_to(b_img.astype(BF), (128, H)).copy()
    eye_bf = np.eye(128, dtype=f32).astype(BF)
    eye_f32 = np.eye(128, dtype=f32)

    in_maps = []
    for c in range(NC):
        r = rows(c)
        W1c = W_ih1[r]
        hr = slice(c * HC, (c + 1) * HC)
        in_maps.append({
            "whh1T": _kchunks(W_hh1[r].T.astype(BF), G),
            "w2T": _kchunks(W2[r].T.astype(BF), G),
            "woutT": _kchunks(W_out[c * VC:(c + 1) * VC].T.astype(BF), VC),
            "weT": _kchunks(W1c[:, :E].T.astype(BF), G),
            "wfT": _kchunks(W1c[:, E:].T.astype(BF), G),
            "wimgT": wimgT_np,
            "wimgcT": _kchunks(W_img[hr].T.astype(BF), 128),
            "ivT": ivT_np,
            "capT": capT_np,
            "b1b": np.broadcast_to(b1[r].astype(BF), (128, G)).copy(),
            "b2b": np.broadcast_to(b2[r].astype(BF), (128, G)).copy(),
            "bimgb": bimgb_np,
            "bimgcb": np.broadcast_to(b_img[hr].astype(BF), (128, 128)).copy(),
            "boutb": np.broadcast_to(
                b_out[c * VC:(c + 1) * VC].astype(BF), (128, VC)).copy(),
            "idn": eye_bf,
            "idnf": eye_f32,
        })
    return in_maps


def kernel(**inputs):
    in_maps = _in_maps(**inputs)
    nc = bass.Bass("TRN2", target_bir_lowering=False, debug=False,
                   num_devices=NC)
    _build(nc)
    res = run_bass_kernel_spmd(nc, in_maps, core_ids=list(range(NC)))

    out = np.empty((B, T, V), np.float32)
    for c in range(NC):
        yc = res.results[c]["y"].reshape(T, B, VC)
        out[:, :, c * VC:(c + 1) * VC] = yc.transpose(1, 0, 2)
    return out


# revision 50
# speedup vs baseline: 1.0791x; 1.0339x over previous
"""CaptionNet (attention + 2-LSTM) Trainium2 kernel, 8 NeuronCores.

Exploits:
- attention softmax over a size-1 axis == 1.0 exactly -> context == image_vectors
- LSTM2 uses h1 as input AND state -> W2 = W_ih2 + W_hh2 folds into one matmul
- word-emb / image parts of the LSTM1 input products are precomputed batched

Sharding: H sharded 8-way in the recurrent loop (h chunks exchanged via
AllGather twice/step); vocab projection sharded 8-way over V; weights
pre-transposed/pre-cast to bf16 on the host (pure layout/sharding work).

Perf structure (vs the naive version):
- AllGather output is pulled into SBUF as 8 per-k-chunk DMAs spread over
  4 engines' queues so the gate matmul starts ~0.4us after the AG lands
  and consumes chunks as they arrive.
- The SBUF->DRAM AG-input DMA is split across two engines' queues.
- The eltwise chain computes sigmoid(f) first so the DVE c-update starts
  while the other gate activations still run.
- pre-tiles (emb@W + img + b1) for step t+2 are computed inside step t's
  AG wait window instead of in the preamble.
- Dummy matmuls fill the PE idle window during each AG to keep the HAM
  clock gate at 8/8 (PE cold costs 2x on every matmul otherwise).
- Two throwaway AllGathers run during the preamble to warm the ncfw
  collective path before the first real (latency-critical) AG.
"""

import contextlib
import numpy as np
import ml_dtypes
import concourse.bass as bass
import concourse.mybir as mybir
from concourse.bass_utils import run_bass_kernel_spmd

B, T, V, E, H, F = 128, 24, 12000, 512, 1024, 2048
NC = 8
HC = H // NC          # 128
G = 4 * HC            # 512 gate rows per core (i,f,o,g reordered)
VC = V // NC          # 1500
KT = H // 128         # 8
ET = E // 128         # 4
FT = F // 128         # 16
F32 = mybir.dt.float32
BF16 = mybir.dt.bfloat16
AF = mybir.ActivationFunctionType
ALU = mybir.AluOpType
BF = ml_dtypes.bfloat16

# out-proj column split (PSUM banks are 512 f32 wide)
OPN = [(0, 512), (512, 512), (1024, VC - 1024)]
NTMR = 15    # DVE timer copies per AG window (~0.85us each) pacing the
NDUM = 3     # keep-warm dummy-matmul batches (NDUM matmuls per 2 timers)


def _kchunks(wT, n_free):
    """[K, n] -> [128, (K//128)*n]; [p, k*n+j] = wT[k*128+p, j]."""
    K = wT.shape[0]
    return np.ascontiguousarray(
        wT.reshape(K // 128, 128, n_free).transpose(1, 0, 2).reshape(128, -1))


def _build(nc):
    def inp(name, shape, dt):
        return nc.dram_tensor(name, list(shape), dt, kind="ExternalInput").ap()

    whh1T = inp("whh1T", [128, KT * G], BF16).rearrange("p (k g) -> p k g", k=KT)
    w2T = inp("w2T", [128, KT * G], BF16).rearrange("p (k g) -> p k g", k=KT)
    woutT = inp("woutT", [128, KT * VC], BF16).rearrange("p (k v) -> p k v", k=KT)
    weT = inp("weT", [128, ET * G], BF16).rearrange("p (k g) -> p k g", k=ET)
    wfT = inp("wfT", [128, FT * G], BF16).rearrange("p (k g) -> p k g", k=FT)
    wimgT = inp("wimgT", [128, FT * H], BF16).rearrange("p (k h) -> p k h", k=FT)
    wimgcT = inp("wimgcT", [128, FT * 128], BF16).rearrange(
        "p (k h) -> p k h", k=FT)
    ivT = inp("ivT", [128, FT * 128], BF16).rearrange("p (k b) -> p k b", k=FT)
    capT = inp("capT", [128, ET * B * T], BF16).rearrange(
        "p (k n) -> p k n", k=ET)
    b1b = inp("b1b", [128, G], BF16)
    b2b = inp("b2b", [128, G], BF16)
    bimgb = inp("bimgb", [128, H], BF16)
    bimgcb = inp("bimgcb", [128, 128], BF16)
    boutb = inp("boutb", [128, VC], BF16)
    idn = inp("idn", [128, 128], BF16)
    idnf = inp("idnf", [128, 128], F32)
    y = nc.dram_tensor("y", [B * T, VC], F32, kind="ExternalOutput").ap()
    # AG buffers declared flat so ncfw's SDMA descriptors cover large
    # contiguous runs (2D [128,256B] shapes made the AG desc-rate-bound).
    bin1 = nc.dram_tensor("bin1", [1, 128 * B], BF16, kind="Internal").ap()
    bout1 = nc.dram_tensor("bout1", [1, H * B], BF16, kind="Internal",
                           addr_space="Shared").ap()
    bin2 = nc.dram_tensor("bin2", [1, 128 * B], BF16, kind="Internal").ap()
    bout2 = nc.dram_tensor("bout2", [1, H * B], BF16, kind="Internal",
                           addr_space="Shared").ap()
    bin1v = bin1.rearrange("a (p b) -> (a p) b", b=B)          # [128, B]
    bin2v = bin2.rearrange("a (p b) -> (a p) b", b=B)
    bout1v = bout1.rearrange("a (k p b) -> (a p) k b", k=KT, b=B)  # [128,8,B]
    bout2v = bout2.rearrange("a (k p b) -> (a p) k b", k=KT, b=B)

    PE, ACT, DVE, SP, PL = nc.tensor, nc.scalar, nc.vector, nc.sync, nc.gpsimd
    ctx = contextlib.ExitStack()
    sb = lambda n, s, d: ctx.enter_context(nc.sbuf_tensor(n, s, d))
    ps = lambda n, s, d: ctx.enter_context(nc.psum_tensor(n, s, d))
    sem = lambda n: ctx.enter_context(nc.semaphore(n))

    # persistent SBUF
    s_whh1 = sb("s_whh1", [128, KT, G], BF16)
    s_w2 = sb("s_w2", [128, KT, G], BF16)
    s_wout = sb("s_wout", [128, KT, VC], BF16)
    s_pre = sb("s_pre", [128, T, G], BF16)
    s_cap = sb("s_cap", [128, ET, B * T], BF16)
    s_we = sb("s_we", [128, ET, G], BF16)
    s_img = sb("s_img", [128, G], BF16)
    s_b2b = sb("s_b2b", [128, G], BF16)
    s_boutb = sb("s_boutb", [128, VC], BF16)
    s_idn = sb("s_idn", [128, 128], BF16)
    s_idnf = sb("s_idnf", [128, 128], F32)
    s_h1T = sb("s_h1T", [128, KT, 128], BF16)
    s_h2T = [sb(f"s_h2T{i}", [128, KT, 128], BF16) for i in range(2)]
    s_c = sb("s_c", [128, HC], F32)
    s_sig = sb("s_sig", [128, 384], F32)
    s_tg = sb("s_tg", [128, HC], F32)
    s_th = sb("s_th", [128, HC], F32)
    s_tA = sb("s_tA", [128, HC], F32)
    s_tB = sb("s_tB", [128, HC], F32)
    s_h = sb("s_h", [128, HC], BF16)  # bf16: single-pass PE transpose
    s_hcT1 = sb("s_hcT1", [128, 128], BF16)
    s_hcT2 = sb("s_hcT2", [128, 128], BF16)
    s_out = sb("s_out", [128, VC], F32)
    s_tsrc = sb("s_tsrc", [128, 1024], F32)  # DVE keep-warm timer scratch
    s_tdst = sb("s_tdst", [128, 1024], F32)

    s_ld = sem("s_ld")
    s_warm = sem("s_warm")
    s_bh0 = sem("s_bh0"); s_bh0ev = sem("s_bh0ev")
    s_bimg = sem("s_bimg"); s_bimgev = sem("s_bimgev")
    s_bc0 = sem("s_bc0"); s_bc0ev = sem("s_bc0ev")
    s_bh0T = sem("s_bh0T"); s_bh0Tev = sem("s_bh0Tev")
    s_preMM = sem("s_preMM"); s_preEv = sem("s_preEv")
    s_gd = sem("s_gd")                      # g1/g2 drains, +2/step
    s_sf = sem("s_sf"); s_si = sem("s_si")  # eltwise ACT milestones, +2/step
    s_cc = sem("s_cc"); s_thS = sem("s_thS")
    s_hh = sem("s_hh")                      # h produced, +2/step
    s_tp = sem("s_tp"); s_ev = sem("s_ev")  # transpose + its evac, +2/step
    s_do = sem("s_do")                      # dma-out halves, +64/step
    s_ag1 = sem("s_ag1"); s_ag2 = sem("s_ag2")
    s_cSP = sem("s_cSP"); s_cAC = sem("s_cAC")  # pull sems (+16/phase each)
    s_tmv = sem("s_tmv")  # DVE timer ticks, +2*NTMR/step
    s_op = sem("s_op"); s_oev = sem("s_oev"); s_odma = sem("s_odma")

    n_ld = 0
    def load(dst, src):
        nonlocal n_ld
        SP.dma_start(dst, src).then_inc(s_ld, 16)
        n_ld += 16

    load(s_whh1[:], whh1T)
    load(s_w2[:], w2T)
    load(s_wout[:], woutT)
    load(s_cap[:], capT)
    load(s_we[:], weT)
    load(s_b2b[:], b2b)
    load(s_boutb[:], boutb)
    load(s_idn[:], idn)
    load(s_idnf[:], idnf)

    # warm the ncfw collective path during the preamble (results unused)
    PL.collective_compute(
        "AllGather", ALU.bypass, replica_groups=[list(range(NC))],
        ins=[bin1.opt()], outs=[bout1.opt()]).then_inc(s_warm, 1)
    PL.collective_compute(
        "AllGather", ALU.bypass, replica_groups=[list(range(NC))],
        ins=[bin2.opt()], outs=[bout2.opt()]).then_inc(s_warm, 1)

    # AG output pulled 3 ways (k0-2 SP, k3-5 PL, k6-7 ACT); each engine
    # does one pull per AG phase: +32/step on its sem.
    def half_thresh(t, phase):
        return 32 * t + (32 if phase == 2 else 16)

    # ---------------- preamble ----------------
    with (
        nc.sbuf_tensor("s_wimg", [128, FT, H], BF16) as s_wimg,
        nc.sbuf_tensor("s_wimgc", [128, FT, 128], BF16) as s_wimgc,
        nc.sbuf_tensor("s_ivT", [128, FT, 128], BF16) as s_ivT,
        nc.sbuf_tensor("s_wf", [128, FT, G], BF16) as s_wf,
        nc.sbuf_tensor("s_b1b", [128, G], BF16) as s_b1b,
        nc.sbuf_tensor("s_bimgb", [128, H], BF16) as s_bimgb,
        nc.sbuf_tensor("s_bimgcb", [128, 128], BF16) as s_bimgcb,
        nc.sbuf_tensor("s_h0", [128, H], F32) as s_h0,
        nc.psum_tensor("p_h0", [128, H], F32) as p_h0,
        nc.psum_tensor("p_pre0", [128, 4, G], F32) as p_pre0,
    ):
        load(s_wimg[:], wimgT)
        load(s_wimgc[:], wimgcT)
        load(s_ivT[:], ivT)
        load(s_wf[:], wfT)
        load(s_b1b[:], b1b)
        load(s_bimgb[:], bimgb)
        load(s_bimgcb[:], bimgcb)
        PE.wait_ge(s_ld, n_ld)

        # h0 = IV @ W_img.T + b_img (replicated full)
        for nn2 in range(2):
            sl = slice(nn2 * 512, (nn2 + 1) * 512)
            for k in range(FT):
                PE.matmul(p_h0[:, sl], s_ivT[:, k, :], s_wimg[:, k, sl],
                          start=(k == 0), stop=False)
            PE.matmul(p_h0[:, sl], s_idn[:], s_bimgb[:, sl],
                      start=False, stop=True)
        PE.drain().then_inc(s_bh0, 1)
        DVE.wait_ge(s_bh0, 1)
        DVE.tensor_copy(s_h0[:], p_h0[:, :]).then_inc(s_bh0ev, 1)

        # c0 chunk = IV @ W_img[chunk].T + b_img[chunk]
        for k in range(FT):
            PE.matmul(p_pre0[:, 3, 0:128], s_ivT[:, k, :], s_wimgc[:, k, :],
                      start=(k == 0), stop=False)
        PE.matmul(p_pre0[:, 3, 0:128], s_idn[:], s_bimgcb[:],
                  start=False, stop=True)
        PE.drain().then_inc(s_bc0, 1)
        DVE.wait_ge(s_bc0, 1)
        DVE.tensor_copy(s_c[:], p_pre0[:, 3, 0:128]).then_inc(s_bc0ev, 1)

        # img_part = IV @ WF_c.T + b1  (bank 0)
        for k in range(FT):
            PE.matmul(p_pre0[:, 0, :], s_ivT[:, k, :], s_wf[:, k, :],
                      start=(k == 0), stop=False)
        PE.matmul(p_pre0[:, 0, :], s_idn[:], s_b1b[:], start=False, stop=True)
        PE.drain().then_inc(s_bimg, 1)
        ACT.wait_ge(s_bimg, 1)
        ACT.activation(s_img[:], p_pre0[:, 0, :], AF.Copy).then_inc(s_bimgev, 1)

        # h0T chunks -> s_h2T[1]  (bank 1, serialized via evac sem)
        PE.wait_ge(s_bh0ev, 1)
        for k in range(KT):
            if k > 0:
                PE.wait_ge(s_bh0Tev, k)
            PE.transpose(p_pre0[:, 1, 0:128], s_h0[:, k * 128:(k + 1) * 128],
                         s_idnf[:])
            PE.drain().then_inc(s_bh0T, 1)
            DVE.wait_ge(s_bh0T, k + 1)
            DVE.tensor_copy(s_h2T[1][:, k, :], p_pre0[:, 1, 0:128]).then_inc(
                s_bh0Tev, 1)

        # pre tiles 0 and 1 only (the rest are computed inside the T loop)
        PE.wait_ge(s_bimgev, 1)
        for m in range(2):
            bank = 0 if (m % 2 == 0) else 2
            for k in range(ET):
                PE.matmul(p_pre0[:, bank, :],
                          s_cap[:, k, m * 128:(m + 1) * 128],
                          s_we[:, k, :], start=(k == 0), stop=False)
            PE.matmul(p_pre0[:, bank, :], s_idn[:], s_img[:],
                      start=False, stop=True)
            PE.drain().then_inc(s_preMM, 1)
            if m % 2 == 0:
                ACT.wait_ge(s_preMM, m + 1)
                ACT.activation(s_pre[:, m, :], p_pre0[:, bank, :],
                               AF.Copy).then_inc(s_preEv, 1)
            else:
                DVE.wait_ge(s_preMM, m + 1)
                DVE.tensor_copy(s_pre[:, m, :], p_pre0[:, bank, :]).then_inc(
                    s_preEv, 1)

    # persistent PSUM
    p_g1 = ps("p_g1", [128, G], F32)
    p_g2 = ps("p_g2", [128, G], F32)
    p_t1 = ps("p_t1", [128, 128], BF16)
    p_t2 = ps("p_t2", [128, 128], BF16)
    p_op = ps("p_op", [128, 1536], F32)
    p_pre = ps("p_pre", [128, G], F32)

    def eltwise(psrc, phase_i):
        """phase_i = 2t+1 (lstm1) or 2t+2 (lstm2); reads psrc, updates s_c,
        writes s_h. f-gate sigmoid first so the DVE chain starts early."""
        ACT.wait_ge(s_gd, phase_i)
        ACT.activation(s_sig[:, 128:256], psrc[:, 128:256],
                       AF.Sigmoid).then_inc(s_sf, 1)
        ACT.activation(s_tg[:], psrc[:, 384:512], AF.Tanh)
        ACT.activation(s_sig[:, 0:128], psrc[:, 0:128],
                       AF.Sigmoid).then_inc(s_si, 1)
        ACT.activation(s_sig[:, 256:384], psrc[:, 256:384], AF.Sigmoid)
        DVE.wait_ge(s_sf, phase_i)
        DVE.scalar_tensor_tensor(s_tA[:], s_sig[:, 128:256], 1.0, s_c[:],
                                 ALU.mult, ALU.mult)
        DVE.wait_ge(s_si, phase_i)
        DVE.scalar_tensor_tensor(s_tB[:], s_sig[:, 0:128], 1.0, s_tg[:],
                                 ALU.mult, ALU.mult)
        DVE.scalar_tensor_tensor(s_c[:], s_tA[:], 1.0, s_tB[:],
                                 ALU.mult, ALU.add).then_inc(s_cc, 1)
        ACT.wait_ge(s_cc, phase_i)
        ACT.activation(s_th[:], s_c[:], AF.Tanh).then_inc(s_thS, 1)
        DVE.wait_ge(s_thS, phase_i)
        DVE.scalar_tensor_tensor(s_h[:], s_sig[:, 256:384], 1.0, s_th[:],
                                 ALU.mult, ALU.mult).then_inc(s_hh, 1)

    def op_chunks(h2Tprev, ks, start):
        for k in ks:
            for (o, w) in OPN:
                PE.matmul(p_op[:, o:o + w], h2Tprev[:, k, :],
                          s_wout[:, k, o:o + w],
                          start=(start and k == ks[0]), stop=False)

    for t in range(T):
        h2buf = s_h2T[(t - 1) % 2]   # holds h2(t-1); h0 lives in buf 1
        ph1, ph2 = 2 * t + 1, 2 * t + 2

        # ---- PE: g1(t), consuming AG2(t-1) pulls as they land ----
        for k in range(KT):
            if t > 0:
                if k == 0:
                    PE.wait_ge(s_cSP, half_thresh(t - 1, 2))
                elif k == 4:
                    PE.wait_ge(s_cAC, half_thresh(t - 1, 2))
            elif k == 0:
                PE.wait_ge(s_bh0Tev, KT)
                PE.wait_ge(s_bc0ev, 1)
                PE.wait_ge(s_preEv, 1)
            PE.matmul(p_g1[:], h2buf[:, k, :], s_whh1[:, k, :],
                      start=(k == 0), stop=False)
        PE.matmul(p_g1[:], s_idn[:], s_pre[:, t, :], start=False, stop=True)
        PE.drain().then_inc(s_gd, 1)

        eltwise(p_g1, ph1)

        # ---- PE: out-proj(t-1) part 1 (covers the eltwise wait) ----
        if t > 0:
            PE.wait_ge(s_oev, t - 1)
            op_chunks(h2buf, [0, 1], start=True)

        # ---- PE: transpose h1 -> p_t1; ACT evacuates ----
        PE.wait_ge(s_hh, ph1)
        PE.transpose(p_t1[:], s_h[:], s_idn[:])
        PE.drain().then_inc(s_tp, 1)
        ACT.wait_ge(s_tp, ph1)
        ACT.activation(s_hcT1[:], p_t1[:], AF.Copy).then_inc(s_ev, 1)

        # ---- SP + ACT: split AG1 input store; PL: doorbell ----
        SP.wait_ge(s_ev, ph1)
        if t > 0:
            SP.wait_ge(s_ag1, t)
        SP.dma_start(bin1v[0:64, :], s_hcT1[0:64, :]).then_inc(s_do, 16)
        if t > 0:
            ACT.wait_ge(s_ag1, t)
        ACT.dma_start(bin1v[64:128, :], s_hcT1[64:128, :]).then_inc(s_do, 16)
        PL.wait_ge(s_do, 64 * t + 32)
        if t == 0:
            PL.wait_ge(s_warm, 2)
        PL.collective_compute(
            "AllGather", ALU.bypass, replica_groups=[list(range(NC))],
            ins=[bin1.opt()], outs=[bout1.opt()]).then_inc(s_ag1, 1)

        # ---- PE window 1 fill: rest of op A, pre(t+2), keep-warm ----
        if t > 0:
            op_chunks(h2buf, [2, 3], start=False)
        if t + 2 < T:
            PE.wait_ge(s_preEv, t + 2)  # WAW: previous tile evacuated
            for k in range(ET):
                PE.matmul(p_pre[:], s_cap[:, k, (t + 2) * 128:(t + 3) * 128],
                          s_we[:, k, :], start=(k == 0), stop=False)
            PE.matmul(p_pre[:], s_idn[:], s_img[:], start=False, stop=True)
            PE.drain().then_inc(s_preMM, 1)
            DVE.wait_ge(s_preMM, t + 3)  # 2 preamble drains + t+1 in-loop
            DVE.tensor_copy(s_pre[:, t + 2, :], p_pre[:]).then_inc(s_preEv, 1)
        # DVE timer chain paces keep-warm dummy matmuls through the AG wait
        for j in range(NTMR):
            DVE.tensor_copy(s_tdst[:], s_tsrc[:]).then_inc(s_tmv, 1)
        for j in range(NTMR // 2):
            PE.wait_ge(s_tmv, 2 * NTMR * t + 2 * (j + 1))
            for _ in range(NDUM):
                PE.matmul(p_g1[:, 0:128], s_idn[:], s_idn[:], start=True, stop=True)

        # ---- AG1 pulls: k0-3 SP, k4-7 ACT ----
        SP.wait_ge(s_ag1, t + 1)
        SP.dma_start(s_h1T[:, 0:4, :], bout1v[:, 0:4, :]).then_inc(s_cSP, 16)
        ACT.wait_ge(s_ag1, t + 1)
        ACT.dma_start(s_h1T[:, 4:8, :], bout1v[:, 4:8, :]).then_inc(s_cAC, 16)

        # ---- PE: g2(t) ----
        for k in range(KT):
            if k == 0:
                PE.wait_ge(s_cSP, half_thresh(t, 1))
            elif k == 4:
                PE.wait_ge(s_cAC, half_thresh(t, 1))
            PE.matmul(p_g2[:], s_h1T[:, k, :], s_w2[:, k, :],
                      start=(k == 0), stop=False)
        PE.matmul(p_g2[:], s_idn[:], s_b2b[:], start=False, stop=True)
        PE.drain().then_inc(s_gd, 1)

        eltwise(p_g2, ph2)

        # ---- PE: out-proj(t-1) part 2 start (covers eltwise2) ----
        if t > 0:
            op_chunks(h2buf, [4, 5], start=False)

        # ---- PE: transpose h2 -> p_t2; ACT evacuates ----
        PE.wait_ge(s_hh, ph2)
        PE.transpose(p_t2[:], s_h[:], s_idn[:])
        PE.drain().then_inc(s_tp, 1)
        ACT.wait_ge(s_tp, ph2)
        ACT.activation(s_hcT2[:], p_t2[:], AF.Copy).then_inc(s_ev, 1)

        # ---- SP + ACT: split AG2 input store; PL: doorbell ----
        SP.wait_ge(s_ev, ph2)
        if t > 0:
            SP.wait_ge(s_ag2, t)
        SP.dma_start(bin2v[0:64, :], s_hcT2[0:64, :]).then_inc(s_do, 16)
        if t > 0:
            ACT.wait_ge(s_ag2, t)
        ACT.dma_start(bin2v[64:128, :], s_hcT2[64:128, :]).then_inc(s_do, 16)
        PL.wait_ge(s_do, 64 * (t + 1))
        PL.collective_compute(
            "AllGather", ALU.bypass, replica_groups=[list(range(NC))],
            ins=[bin2.opt()], outs=[bout2.opt()]).then_inc(s_ag2, 1)

        # ---- PE window 2 fill: rest of op B + bias, keep-warm ----
        if t > 0:
            op_chunks(h2buf, [6, 7], start=False)
            for (o, w) in OPN:
                PE.matmul(p_op[:, o:o + w], s_idn[:], s_boutb[:, o:o + w],
                          start=False, stop=True)
            PE.drain().then_inc(s_op, 1)
            ACT.wait_ge(s_op, t)
            if t > 1:
                ACT.wait_ge(s_odma, 16 * (t - 1))
            ACT.activation(s_out[:], p_op[:, 0:VC], AF.Copy).then_inc(s_oev, 1)
            SP.wait_ge(s_oev, t)
            SP.dma_start(y[(t - 1) * 128:t * 128, :], s_out[:]).then_inc(
                s_odma, 16)
        for j in range(NTMR):
            DVE.tensor_copy(s_tdst[:], s_tsrc[:]).then_inc(s_tmv, 1)
        for j in range(NTMR // 2):
            PE.wait_ge(s_tmv, 2 * NTMR * t + NTMR + 2 * (j + 1))
            for _ in range(NDUM):
                PE.matmul(p_g2[:, 0:128], s_idn[:], s_idn[:], start=True, stop=True)

        # ---- AG2 pulls into s_h2T[t % 2] ----
        dstbuf = s_h2T[t % 2]
        SP.wait_ge(s_ag2, t + 1)
        SP.dma_start(dstbuf[:, 0:4, :], bout2v[:, 0:4, :]).then_inc(s_cSP, 16)
        ACT.wait_ge(s_ag2, t + 1)
        ACT.dma_start(dstbuf[:, 4:8, :], bout2v[:, 4:8, :]).then_inc(s_cAC, 16)

    # epilogue: out-proj for t = T-1
    PE.wait_ge(s_oev, T - 1)
    PE.wait_ge(s_cSP, half_thresh(T - 1, 2))
    PE.wait_ge(s_cAC, half_thresh(T - 1, 2))
    last = s_h2T[(T - 1) % 2]
    for k in range(KT):
        for (o, w) in OPN:
            PE.matmul(p_op[:, o:o + w], last[:, k, :], s_wout[:, k, o:o + w],
                      start=(k == 0), stop=False)
    for (o, w) in OPN:
        PE.matmul(p_op[:, o:o + w], s_idn[:], s_boutb[:, o:o + w],
                  start=False, stop=True)
    PE.drain().then_inc(s_op, 1)
    ACT.wait_ge(s_op, T)
    ACT.wait_ge(s_odma, 16 * (T - 1))
    ACT.activation(s_out[:], p_op[:, 0:VC], AF.Copy).then_inc(s_oev, 1)
    SP.wait_ge(s_oev, T)
    SP.dma_start(y[(T - 1) * 128:T * 128, :], s_out[:]).then_inc(s_odma, 16)
    SP.wait_ge(s_odma, 16 * T)

    ctx.close()
    return nc


def _in_maps(image_vectors, captions_ix, W_img, b_img, emb, Wa, ba, Ua, ub,
             va, vb, W_ih1, W_hh1, b_ih1, b_hh1, W_ih2, W_hh2, b_ih2, b_hh2,
             W_out, b_out):
    f32 = np.float32
    IV = np.asarray(image_vectors, f32)
    cap = np.asarray(captions_ix).astype(np.int64)
    W_ih1 = np.asarray(W_ih1, f32); W_hh1 = np.asarray(W_hh1, f32)
    W2 = np.asarray(W_ih2, f32) + np.asarray(W_hh2, f32)
    b1 = np.asarray(b_ih1, f32) + np.asarray(b_hh1, f32)
    b2 = np.asarray(b_ih2, f32) + np.asarray(b_hh2, f32)
    W_out = np.asarray(W_out, f32); b_out = np.asarray(b_out, f32)
    W_img = np.asarray(W_img, f32); b_img = np.asarray(b_img, f32)
    emb_ = np.asarray(emb, f32)

    def rows(c):
        idx = []
        for gsel in (0, 1, 3, 2):  # torch (i,f,g,o) -> ours (i,f,o,g)
            base = gsel * H + c * HC
            idx.extend(range(base, base + HC))
        return np.array(idx)

    ce = emb_[cap.T.reshape(-1)]                       # [(t,b), E]
    capT_np = _kchunks(ce.T.astype(BF), B * T)
    ivT_np = _kchunks(IV.T.astype(BF), 128)
    wimgT_np = _kchunks(W_img.T.astype(BF), H)
    bimgb_np = np.broadcast_to(b_img.astype(BF), (128, H)).copy()
    eye_bf = np.eye(128, dtype=f32).astype(BF)
    eye_f32 = np.eye(128, dtype=f32)

    in_maps = []
    for c in range(NC):
        r = rows(c)
        W1c = W_ih1[r]
        hr = slice(c * HC, (c + 1) * HC)
        in_maps.append({
            "whh1T": _kchunks(W_hh1[r].T.astype(BF), G),
            "w2T": _kchunks(W2[r].T.astype(BF), G),
            "woutT": _kchunks(W_out[c * VC:(c + 1) * VC].T.astype(BF), VC),
            "weT": _kchunks(W1c[:, :E].T.astype(BF), G),
            "wfT": _kchunks(W1c[:, E:].T.astype(BF), G),
            "wimgT": wimgT_np,
            "wimgcT": _kchunks(W_img[hr].T.astype(BF), 128),
            "ivT": ivT_np,
            "capT": capT_np,
            "b1b": np.broadcast_to(b1[r].astype(BF), (128, G)).copy(),
            "b2b": np.broadcast_to(b2[r].astype(BF), (128, G)).copy(),
            "bimgb": bimgb_np,
            "bimgcb": np.broadcast_to(b_img[hr].astype(BF), (128, 128)).copy(),
            "boutb": np.broadcast_to(
                b_out[c * VC:(c + 1) * VC].astype(BF), (128, VC)).copy(),
            "idn": eye_bf,
            "idnf": eye_f32,
        })
    return in_maps


def kernel(**inputs):
    in_maps = _in_maps(**inputs)
    nc = bass.Bass("TRN2", target_bir_lowering=False, debug=False,
                   num_devices=NC)
    _build(nc)
    res = run_bass_kernel_spmd(nc, in_maps, core_ids=list(range(NC)))

    out = np.empty((B, T, V), np.float32)
    for c in range(NC):
        yc = res.results[c]["y"].reshape(T, B, VC)
        out[:, :, c * VC:(c + 1) * VC] = yc.transpose(1, 0, 2)
    return out
